# revision 24
# baseline (speedup 1.0000x reference)
# Trainium2 Bass kernel for nn_DeepHamCritic (3-layer GATv2 + MLP head).
#
# Strategy (8 NeuronCores, dst-node sharded):
#  - Nodes are assigned to 80 (core, block) bins of 128 slots by balanced
#    binning on degree, so per-bin edge counts are nearly equal and the
#    uniform tile count T is minimal. Edges (incl. self-loops) are routed to
#    the bin owning dst and sorted by local dst rank.
#  - Weights are pre-scaled by |a| (clamped): u' = |a| (.) (xs + xt + b).
#    With the pos-first column permutation, the GATv2 edge score reduces to
#      e = sum_{pos cols} LeakyReLU(u', 0.2) - sum_{neg cols} LeakyReLU(u', 0.2)
#    exactly.  The LeakyReLU runs on the Scalar engine while staging u' from
#    PSUM to SBUF fp16; the two range sums are per-BLOCK 3D-AP tensor_reduce
#    ops on the Vector engine ([128, T, range] -> [128, T]) instead of
#    per-tile reduces.
#  - Both one-hot operators (dst staircase for the xt broadcast, edge->dst
#    one-hot for aggregation) are precomputed on the host and streamed from
#    DRAM per block; exp(e) is folded into the aggregation one-hot with one
#    tensor_scalar per tile (per-partition scalar ex).
#  - Source-row gathers (3 x <=768-row dma_gather chunks per block) are
#    prefetched one block ahead at the top of the loop so the GpSimd
#    descriptor-generation chain (~6.6us per chunk, the pacing engine) runs
#    as early as possible.  AllGather halves fire mid-loop (after node
#    blocks 4/9) to hide the collective behind the previous layer's edge
#    phase.
#  - Everything 16-bit is fp16 (not bf16) for the extra mantissa bits; all
#    matmul accumulation is fp32 in PSUM.
import math
import numpy as np

F16 = np.float16

# ---------------- problem configuration (hardcoded per the contract) --------
FULL_CFG = dict(
    NC=8,            # cores
    NPC=1250,        # real nodes per core
    NBLK=10,         # dst blocks of 128 per core
    D=512,           # GAT hidden width
    DIN=128,         # input feature width
    N_NODES=10000,
)
BLK = 128
A_CLAMP = 2e-4       # clamp |a| to keep fp16-scaled features out of denormals
NSWQ = 1             # SWDGE queues (trigger_dma only fires queue 0's ring)
PREP_GATHER = False  # prepare_only+trigger path (queue 0 only)

_BUILD_CACHE = {}


# ---------------- host-side preprocessing -----------------------------------
def _prep_layer(Ws, bs, Wt, bt, a, c, prev_perm):
    a64 = np.asarray(a, np.float64)
    Ws = np.asarray(Ws, np.float64)
    Wt = np.asarray(Wt, np.float64)
    bs = np.asarray(bs, np.float64)
    bt = np.asarray(bt, np.float64)
    c = np.asarray(c, np.float64)
    if prev_perm is not None:
        Ws = Ws[prev_perm, :]
        Wt = Wt[prev_perm, :]
    pos = a64 > 0
    p = int(pos.sum())
    perm = np.argsort(~pos, kind="stable")
    a_abs = np.maximum(np.abs(a64), A_CLAMP)
    return dict(
        Ws_s=(Ws * a_abs[None, :])[:, perm].astype(F16),
        Wt_s=(Wt * a_abs[None, :])[:, perm].astype(F16),
        bst=((bs + bt) * a_abs)[perm].astype(F16),
        inv_sa=(1.0 / a_abs)[perm].astype(np.float32),
        c_eff=(c + bs)[perm].astype(np.float32),
        p=p, perm=perm,
    )


def _prep_graph(edge_index, cfg):
    NC, NPC, NBLK = cfg["NC"], cfg["NPC"], cfg["NBLK"]
    NP_PAD = NBLK * BLK
    n = cfg["N_NODES"]
    NBINS = NC * NBLK
    CAP = n // NBINS                      # 125 real nodes per bin
    e_src = np.asarray(edge_index[0], np.int64)
    e_dst = np.asarray(edge_index[1], np.int64)

    # balanced binning: assign nodes (weight = in-degree + 1 self-loop) to
    # the lightest of the 80 bins that still has node capacity.
    wt = np.bincount(e_dst, minlength=n).astype(np.int64) + 1
    order = np.argsort(-wt, kind="stable")
    import heapq
    heap = [(0, b) for b in range(NBINS)]   # (edge load, bin)
    heapq.heapify(heap)
    bin_of = np.empty(n, np.int32)
    rank_of = np.empty(n, np.int32)
    bin_fill = np.zeros(NBINS, np.int32)
    for node in order:
        load, b = heapq.heappop(heap)
        bin_of[node] = b
        rank_of[node] = bin_fill[b]
        bin_fill[b] += 1
        if bin_fill[b] < CAP:
            heapq.heappush(heap, (load + int(wt[node]), b))
    slot_of = bin_of.astype(np.int64) * BLK + rank_of   # global padded row id

    loops = np.arange(n, dtype=np.int64)
    src = np.concatenate([e_src, loops])
    dst = np.concatenate([e_dst, loops])
    dslot = slot_of[dst]
    order_e = np.argsort(dslot, kind="stable")   # sorts by (bin, rank)
    src, dslot = src[order_e], dslot[order_e]

    binid = dslot // BLK
    counts = np.bincount(binid, minlength=NBINS)
    T = int(math.ceil(counts.max() / BLK))
    E_BLK = T * BLK
    starts = np.concatenate([[0], np.cumsum(counts)])

    # gather-table row layout for half-split AllGathers: half A (local rows
    # 0..639 of every core) occupies rows [0, 8*640), half B the rest.
    HALF = NP_PAD // 2
    def gather_row(s):
        c = s // NP_PAD
        r = s % NP_PAD
        return np.where(r < HALF, c * HALF + r,
                        NC * HALF + c * HALF + (r - HALF))
    remap = gather_row(slot_of[src])
    nloc = dslot % BLK

    idx = np.zeros((NC, NBLK * E_BLK), np.int16)
    dstl = np.full((NC, NBLK * T, BLK), -1, np.int32)
    lo = np.zeros((NC, NBLK * T, BLK), np.int32)
    hi = np.zeros((NC, NBLK * T, BLK), np.int32)
    ar = np.arange(BLK)
    for cc in range(NC):
        for b in range(NBLK):
            k = cc * NBLK + b
            s0, s1 = starts[k], starts[k + 1]
            cnt = s1 - s0
            base = b * E_BLK
            idx[cc, base:base + cnt] = remap[s0:s1].astype(np.int16)
            dv = dstl[cc].reshape(NBLK, T * BLK)
            dv[b, :cnt] = nloc[s0:s1]
            nb = nloc[s0:s1]
            startn = np.searchsorted(nb, ar, side="left")
            endn = np.searchsorted(nb, ar, side="right")
            toff = (np.arange(T) * BLK)[:, None]
            lo[cc, b * T:(b + 1) * T] = np.clip(startn[None, :] - toff, 0, BLK)
            hi[cc, b * T:(b + 1) * T] = np.clip(endn[None, :] - toff, 0, BLK)

    # wrap idx into the 16-partition layout, replicated to 128 partitions
    idx16 = np.zeros((NC, 128, NBLK * E_BLK // 16), np.int16)
    for cc in range(NC):
        w = idx[cc].reshape(-1, 16).T            # [16, total/16]
        idx16[cc] = np.tile(w, (8, 1))

    # host-precomputed one-hots (streamed per block on-device):
    #  ohb[n(part), ti, j] = (j >= lo[ti, n]) & (j < hi[ti, n])
    #  oha[j(part), ti, n] = (dstl[ti, j] == n)
    lo_t = lo.transpose(0, 2, 1)                 # [NC, 128(n), NBLK*T]
    hi_t = hi.transpose(0, 2, 1)
    ohb = ((ar[None, None, None, :] >= lo_t[:, :, :, None]) &
           (ar[None, None, None, :] < hi_t[:, :, :, None]))
    oha = (dstl[:, :, :, None] == ar[None, None, None, :])
    oha = oha.transpose(0, 2, 1, 3)              # [NC, 128(j), NBLK*T, 128(n)]
    # concat ohb|oha per block into one stream: [NC, 128, NBLK, 2, T, BLK]
    ohb_r = np.ascontiguousarray(ohb).astype(F16).reshape(NC, BLK, NBLK, T, BLK)
    oha_r = np.ascontiguousarray(oha).astype(F16).reshape(NC, BLK, NBLK, T, BLK)
    ohba = np.stack([ohb_r, oha_r], axis=3)      # [NC, 128, NBLK, 2, T, BLK]
    ohba_f = np.ascontiguousarray(ohba).reshape(NC, BLK, -1)

    return dict(idx16=idx16, ohba=ohba_f, T=T, slot_of=slot_of)


def _host_prep(inputs, cfg):
    NC, NPC, NBLK, D, DIN = (cfg["NC"], cfg["NPC"], cfg["NBLK"], cfg["D"],
                             cfg["DIN"])
    NP_PAD = NBLK * BLK
    g = _prep_graph(inputs["edge_index"], cfg)

    layers = []
    prev_perm = None
    for l in (1, 2, 3):
        P = _prep_layer(inputs[f"W{l}s"], inputs[f"b{l}s"], inputs[f"W{l}t"],
                        inputs[f"b{l}t"], inputs[f"a{l}"], inputs[f"c{l}"],
                        prev_perm)
        layers.append(P)
        prev_perm = P["perm"]

    Wh1 = np.asarray(inputs["Wh1"], np.float32)[prev_perm, :]
    Wh2 = np.asarray(inputs["Wh2"], np.float32)
    Wh3 = np.asarray(inputs["Wh3"], np.float32)
    bh1 = np.asarray(inputs["bh1"], np.float32)
    bh2 = np.asarray(inputs["bh2"], np.float32)
    bh3 = float(np.asarray(inputs["bh3"], np.float32).ravel()[0])

    x = np.asarray(inputs["x"], np.float32)
    KC = D // BLK

    common = {
        "ident": np.eye(BLK, dtype=F16),
        "ones1": np.ones((1, BLK), F16),
        "onescol": np.ones((BLK, 1), F16),
        "Wh1": Wh1.astype(F16), "Wh2": Wh2.astype(F16),
        "Wh3pad": Wh3.astype(F16),
        "bh1c": (0.99 * bh1).reshape(KC, BLK).T.astype(np.float32),
        "bh1s": (0.01 * bh1).reshape(KC, BLK).T.astype(np.float32),
        "bh2c": (0.99 * bh2).reshape(KC, BLK).T.astype(np.float32),
        "bh2s": (0.01 * bh2).reshape(KC, BLK).T.astype(np.float32),
    }
    for li, P in enumerate(layers, start=1):
        common[f"Ws{li}"] = P["Ws_s"]
        common[f"Wt{li}"] = P["Wt_s"]
        common[f"bst{li}"] = P["bst"][None, :]
        common[f"crep{li}"] = np.broadcast_to(P["c_eff"][None, :],
                                              (BLK, D)).copy()
        common[f"irep{li}"] = np.broadcast_to(P["inv_sa"][None, :],
                                              (BLK, D)).copy()

    slot_of = g["slot_of"]
    in_maps = []
    for cc in range(NC):
        xs = np.zeros((NP_PAD, DIN), np.float32)
        sel = (slot_of // NP_PAD) == cc
        xs[slot_of[sel] % NP_PAD] = x[sel]
        m = dict(common)
        m["xT_own"] = np.ascontiguousarray(xs.T).astype(F16)  # [DIN, NP_PAD]
        m["idx16"] = g["idx16"][cc]
        m["ohba"] = g["ohba"][cc]
        in_maps.append(m)

    static = dict(T=g["T"], p=[P["p"] for P in layers], bh3=bh3)
    return in_maps, static, slot_of


# ---------------- Bass program ----------------------------------------------
def _build_nc(cfg, static):
    import concourse.bass as bass
    import concourse.bacc as bacc
    import concourse.tile as tile
    from concourse import mybir

    NC, NPC, NBLK, D, DIN = (cfg["NC"], cfg["NPC"], cfg["NBLK"],
                             cfg["D"], cfg["DIN"])
    NP_PAD = NBLK * BLK
    NTOT = NP_PAD * NC
    T = static["T"]
    KC = D // BLK
    f32 = mybir.dt.float32
    f16 = mybir.dt.float16
    AF = mybir.ActivationFunctionType
    OP = mybir.AluOpType
    AX = mybir.AxisListType

    nc = bacc.Bacc("TRN2", target_bir_lowering=False, debug=False,
                   num_devices=NC)

    # ---- I/O ----
    t_xT = nc.dram_tensor("xT_own", [DIN, NP_PAD], f16, kind="ExternalInput")
    t_idx = nc.dram_tensor("idx16", [128, NBLK * T * 8], mybir.dt.int16,
                           kind="ExternalInput")
    t_ohba = nc.dram_tensor("ohba", [128, NBLK * 2 * T * BLK], f16,
                            kind="ExternalInput")
    t_id = nc.dram_tensor("ident", [BLK, BLK], f16, kind="ExternalInput")
    t_ones = nc.dram_tensor("ones1", [1, BLK], f16, kind="ExternalInput")
    t_onescol = nc.dram_tensor("onescol", [BLK, 1], f16,
                               kind="ExternalInput")
    t_W = {}
    for l in (1, 2, 3):
        dl = DIN if l == 1 else D
        t_W[f"Ws{l}"] = nc.dram_tensor(f"Ws{l}", [dl, D], f16,
                                       kind="ExternalInput")
        t_W[f"Wt{l}"] = nc.dram_tensor(f"Wt{l}", [dl, D], f16,
                                       kind="ExternalInput")
        t_W[f"bst{l}"] = nc.dram_tensor(f"bst{l}", [1, D], f16,
                                        kind="ExternalInput")
        t_W[f"crep{l}"] = nc.dram_tensor(f"crep{l}", [BLK, D], f32,
                                         kind="ExternalInput")
        t_W[f"irep{l}"] = nc.dram_tensor(f"irep{l}", [BLK, D], f32,
                                         kind="ExternalInput")
    for nm, shp, dt_ in (("Wh1", [D, D], f16), ("Wh2", [D, D], f16),
                         ("Wh3pad", [D, 1], f16),
                         ("bh1c", [BLK, KC], f32), ("bh1s", [BLK, KC], f32),
                         ("bh2c", [BLK, KC], f32), ("bh2s", [BLK, KC], f32)):
        t_W[nm] = nc.dram_tensor(nm, shp, dt_, kind="ExternalInput")
    t_y = nc.dram_tensor("y", [BLK, NBLK], f32, kind="ExternalOutput")

    with tile.TileContext(nc) as tc:
        with (
            tc.tile_pool(name="const", bufs=1) as cpool,
            tc.tile_pool(name="w", bufs=2) as wpool,
            tc.tile_pool(name="ht", bufs=4) as htpool,
            tc.tile_pool(name="xt", bufs=2) as xtpool,
            tc.tile_pool(name="xscat", bufs=3) as xcpool,
            tc.tile_pool(name="g", bufs=6) as gpool,
            tc.tile_pool(name="ohb", bufs=2) as obpool,
            tc.tile_pool(name="oha", bufs=2) as oapool,
            tc.tile_pool(name="ohaa", bufs=2) as oaapool,
            tc.tile_pool(name="us", bufs=2) as uspool,
            tc.tile_pool(name="scr", bufs=3) as scrpool,
            tc.tile_pool(name="strip", bufs=3) as stpool,
            tc.tile_pool(name="hb", bufs=2) as hbpool,
            tc.tile_pool(name="psu", bufs=3, space="PSUM") as upool,
            tc.tile_pool(name="psagg", bufs=2, space="PSUM") as aggpool,
            tc.tile_pool(name="pspt", bufs=2, space="PSUM") as ptpool,
            tc.tile_pool(name="dram", bufs=1, space="DRAM") as dpool,
        ):
            # ---- resident constants ----
            def load_const(t, shape):
                tl = cpool.tile(shape, t.dtype, name=t.name + "_sb",
                                tag=t.name + "_sb")
                nc.sync.dma_start(tl[:], t.ap())
                return tl

            ident = load_const(t_id, [BLK, BLK])
            ones1 = load_const(t_ones, [1, BLK])
            onescol = load_const(t_onescol, [BLK, 1])
            idx_sb = load_const(t_idx, [128, NBLK * T * 8])
            xT_sb = cpool.tile([DIN, 1, NP_PAD], f16)
            nc.sync.dma_start(xT_sb[:, 0, :], t_xT.ap())
            negtwo = cpool.tile([BLK, 1], f32)
            nc.vector.memset(negtwo[:], -2.0)
            creps, ireps = {}, {}
            for l in (1, 2, 3):
                creps[l] = load_const(t_W[f"crep{l}"], [BLK, D])
                ireps[l] = load_const(t_W[f"irep{l}"], [BLK, D])

            # gather-drain completion semaphores, one per SWDGE queue.
            # Tile does not clear user sems between executions; clear at
            # program start (first gpsimd instructions, well before any
            # consumer evaluates its wait).
            gsems = [nc.alloc_semaphore(f"gsem{q}") for q in range(NSWQ)]
            for s in gsems:
                nc.gpsimd.sem_clear(s)
            gcnt = [0] * NSWQ      # cumulative expected completion counts

            # DRAM scratch
            ag_in = [dpool.tile([NP_PAD, D], f16, name=f"ag_in{i}")
                     for i in range(3)]
            xs_full = [dpool.tile([NTOT, D], f16, name=f"xs_full{i}")
                       for i in range(3)]

            # ---- program: software-pipelined across layers ----
            HALF = NP_PAD // 2

            def node_block(l, hTsrc, ws, wt, bst, xt_sb, m):
                """xs'/xt' for dst block m of layer l."""
                dl = DIN if l == 1 else D
                kcn = dl // BLK
                ms = slice(m * BLK, (m + 1) * BLK)
                ps_x = upool.tile([BLK, D], f32, tag="u", name="ps_x")
                for k in range(kcn):
                    nc.tensor.matmul(ps_x[:], hTsrc[:, k, ms], ws[:, k, :],
                                     start=(k == 0), stop=(k == kcn - 1))
                xc = xcpool.tile([BLK, D], f16, tag="xscat", name="xc")
                nc.scalar.copy(xc[:], ps_x[:])
                nc.sync.dma_start(ag_in[l - 1][ms, :], xc[:])
                ps_t = upool.tile([BLK, D], f32, tag="u", name="ps_t")
                for k in range(kcn):
                    nc.tensor.matmul(ps_t[:], hTsrc[:, k, ms], wt[:, k, :],
                                     start=(k == 0), stop=False)
                nc.tensor.matmul(ps_t[:], ones1[:, 0:BLK], bst[:],
                                 start=False, stop=True)
                nc.scalar.copy(xt_sb[:, m, :], ps_t[:])

            def load_node_weights(l):
                dl = DIN if l == 1 else D
                kcn = dl // BLK
                ws = wpool.tile([BLK, kcn, D], f16, tag="w", name="ws")
                nc.sync.dma_start(
                    ws[:], t_W[f"Ws{l}"].ap().rearrange("(c p) n -> p c n",
                                                        p=BLK))
                wt = wpool.tile([BLK, kcn, D], f16, tag="w", name="wt")
                nc.sync.dma_start(
                    wt[:], t_W[f"Wt{l}"].ap().rearrange("(c p) n -> p c n",
                                                        p=BLK))
                bst = wpool.tile([1, D], f16, tag="bst", name="bst")
                nc.sync.dma_start(bst[:], t_W[f"bst{l}"].ap())
                return ws, wt, bst

            def ag_half(l, half):
                r0, r1_ = (0, HALF) if half == 0 else (HALF, NP_PAD)
                o0 = NC * HALF * half
                nc.gpsimd.collective_compute(
                    "AllGather", mybir.AluOpType.bypass, replica_groups=rg,
                    ins=[ag_in[l - 1][r0:r1_, :]],
                    outs=[xs_full[l - 1][o0:o0 + NC * HALF, :]])

            # one dma_gather is limited to ~1024 descriptors; chunk the
            # per-block gather into <=6-tile (768-idx) pieces
            NSP = max(1, math.ceil(T / 6))
            CH = math.ceil(T / NSP)
            chunks = []
            t0c = 0
            while t0c < T:
                chunks.append((t0c, min(CH, T - t0c)))
                t0c += CH

            def gather_block(l, b, gq):
                """prep + trigger the source-row gather for (layer l, block
                b) on SWDGE queue gq.  Consumers must gate on gsems[gq]
                reaching the returned count (Tile's auto consumer sync is
                broken for prepare_only preps)."""
                xf = xs_full[l - 1]
                gts = []
                for (c0, cn) in chunks:
                    gtc = gpool.tile([128, CH, D], f16, tag="g",
                                     name=f"gt{c0}")
                    if PREP_GATHER:
                        nc.gpsimd.dma_gather(
                            gtc[:, 0:cn, :], xf[:],
                            idx_sb[:, (b * T + c0) * 8:(b * T + c0 + cn) * 8],
                            cn * BLK, cn * BLK, D,
                            prepare_only=True, sem=gsems[gq], queue_num=gq)
                        gcnt[gq] += 16
                    else:
                        nc.gpsimd.dma_gather(
                            gtc[:, 0:cn, :], xf[:],
                            idx_sb[:, (b * T + c0) * 8:(b * T + c0 + cn) * 8],
                            cn * BLK, cn * BLK, D, queue_num=gq)
                    gts.append(gtc)
                if PREP_GATHER:
                    nc.gpsimd.trigger_dma(count=None, queue_num=gq)
                return (gts, gq, gcnt[gq])

            def stream_onehots(b):
                oh_sb = obpool.tile([128, 2, T, BLK], f16, tag="ohba",
                                    name="oh_sb")
                nc.sync.dma_start(
                    oh_sb[:],
                    t_ohba.ap()[:, b * 2 * T * BLK:(b + 1) * 2 * T * BLK]
                    .rearrange("p (u t j) -> p u t j", u=2, t=T))
                return oh_sb

            def u_phase(l, b, xt_sb, gbundle, oh_sb):
                """u' matmuls + LeakyReLU staging for all tiles of block b."""
                gts, gq, gval = gbundle
                def gslice(t):
                    return gts[t // CH][:, t % CH, :]
                us = uspool.tile([BLK, T, D], f16, tag="us", name="us")
                for t in range(T):
                    u = upool.tile([BLK, D], f32, tag="u")
                    nc.tensor.matmul(u[:], oh_sb[:, 0, t, :],
                                     xt_sb[:, b, :], start=True, stop=False)
                    mm = nc.tensor.matmul(u[:], ident[:], gslice(t),
                                          start=False, stop=True)
                    if t == 0 and PREP_GATHER:
                        # gate the PE on the gather drain; later PE reads of
                        # gts are FIFO-ordered behind this one.
                        mm._wait_ge(gsems[gq], gval)
                    nc.scalar.activation(us[:, t, :], u[:], AF.Prelu,
                                         alpha=0.2)
                return us

            def score_phase(l, us, oh_sb):
                """block-level score reduction + softmax numerators."""
                p = static["p"][l - 1]
                # even-aligned split keeps the DVE reduces in the packed
                # perf mode; a p-odd split is fixed up with the straggler
                # column (counted +1 in ra instead of -1): e = ra - rb -
                # 2*us[:, :, p'].
                p2 = p + (p & 1)
                ex = stpool.tile([BLK, T], f32, tag="ex")
                e0 = stpool.tile([BLK, T], f32, tag="e0")
                ra = stpool.tile([BLK, T], f32, tag="ra")
                nc.vector.tensor_reduce(ra[:], us[:, :, 0:p2], AX.X, OP.add)
                rb = stpool.tile([BLK, T], f32, tag="rb")
                nc.vector.tensor_reduce(rb[:], us[:, :, p2:D], AX.X, OP.add)
                nc.vector.tensor_tensor(e0[:], ra[:], rb[:], OP.subtract)
                if p2 != p:
                    e1 = stpool.tile([BLK, T], f32, tag="e1")
                    nc.vector.scalar_tensor_tensor(
                        e1[:], us[:, :, p:p + 1], -2.0, e0[:],
                        OP.mult, OP.add)
                    e0 = e1
                nc.scalar.activation(ex[:], e0[:], AF.Exp,
                                     bias=negtwo[:], scale=1.0)
                ohaa = oaapool.tile([128, T, BLK], f16, tag="ohaa")
                nc.vector.tensor_tensor(
                    ohaa[:], oh_sb[:, 1, :, :],
                    ex[:].unsqueeze(2).broadcast_to([BLK, T, BLK]),
                    OP.mult)
                return ohaa

            def agg_phase(l, b, gts, ohaa, hT_new):
                """aggregation matmuls + postscale + tanh + transpose-out."""
                def gslice(t):
                    return gts[t // CH][:, t % CH, :]
                agg = aggpool.tile([BLK, D], f32, tag="agg")
                s_ps = ptpool.tile([BLK, 1], f32, tag="s", name="s_ps")
                for t in range(T):
                    nc.tensor.matmul(agg[:], ohaa[:, t, :], gslice(t),
                                     start=(t == 0), stop=(t == T - 1))
                    nc.tensor.matmul(s_ps[:], ohaa[:, t, :], onescol[:],
                                     start=(t == 0), stop=(t == T - 1))
                s_sb = stpool.tile([BLK, 1], f32, tag="s_sb")
                nc.vector.tensor_scalar(s_sb[:], s_ps[:], 1e-30, None,
                                        OP.add)
                rs = stpool.tile([BLK, 1], f32, tag="rs")
                nc.vector.reciprocal_approx_fast(rs[:], s_sb[:])
                t2 = scrpool.tile([BLK, D], f32, tag="t2")
                nc.vector.scalar_tensor_tensor(t2[:], agg[:], rs[:],
                                               ireps[l][:], OP.mult,
                                               OP.mult)
                t3 = scrpool.tile([BLK, D], f32, tag="t3")
                nc.vector.tensor_add(t3[:], t2[:], creps[l][:])
                hb = hbpool.tile([BLK, D], f16, tag="hb")
                nc.scalar.activation(hb[:], t3[:], AF.Tanh)
                # transpose into hT_new via the DMA xbar
                for fc in range(KC):
                    nc.sync.dma_start_transpose(
                        hT_new[:, fc, b * BLK:(b + 1) * BLK],
                        hb[:, fc * BLK:(fc + 1) * BLK])

            def load_mlp_weights(wname, bc, bs_):
                w = wpool.tile([BLK, KC, D], f16, tag="w", name="wh")
                nc.sync.dma_start(
                    w[:], t_W[wname].ap().rearrange("(c p) n -> p c n", p=BLK))
                bcs = wpool.tile([BLK, KC], f32, tag="bc", name="bcs")
                nc.sync.dma_start(bcs[:], t_W[bc].ap())
                bss = wpool.tile([BLK, KC], f32, tag="bs", name="bss")
                nc.sync.dma_start(bss[:], t_W[bs_].ap())
                return w, bcs, bss

            def mlp_block(src, dst, w, bcs, bss, m):
                """One leaky MLP layer for node block m (128 cols)."""
                msl = slice(m * BLK, (m + 1) * BLK)
                for no in range(KC):
                    zp = upool.tile([BLK, BLK], f32, tag="u", name="zp")
                    for k in range(KC):
                        nc.tensor.matmul(
                            zp[:], w[:, k, no * BLK:(no + 1) * BLK],
                            src[:, k, msl], start=(k == 0),
                            stop=(k == KC - 1))
                    rr = scrpool.tile([BLK, BLK], f16, tag="rr")
                    nc.scalar.activation(rr[:], zp[:], AF.Relu,
                                         bias=bcs[:, no:no + 1], scale=0.99)
                    tl = scrpool.tile([BLK, BLK], f32, tag="tl")
                    nc.vector.tensor_scalar(tl[:], zp[:], 0.01,
                                            bss[:, no:no + 1],
                                            OP.mult, OP.add)
                    nc.vector.tensor_add(dst[:, no, msl], tl[:], rr[:])

            rg = [list(range(NC))]

            # ---- layer 1 node phase + first AllGather (halves fired early) --
            ws1, wt1, bst1 = load_node_weights(1)
            xt_cur = xtpool.tile([BLK, NBLK, D], f16, tag="xt", name="xt1")
            for m in range(NBLK):
                node_block(1, xT_sb, ws1, wt1, bst1, xt_cur, m)
                if m == NBLK // 2 - 1:
                    ag_half(1, 0)
            ag_half(1, 1)

            # ---- pipelined layers ----
            hT_prev = xT_sb
            for l in (1, 2, 3):
                gqrr = [((l - 1) * NBLK + b) % NSWQ for b in range(NBLK)]
                hT_new = htpool.tile([BLK, KC, NP_PAD], f16, tag="ht",
                                     name=f"hT{l}")
                if l < 3:
                    wsn, wtn, bstn = load_node_weights(l + 1)
                    xt_next = xtpool.tile([BLK, NBLK, D], f16, tag="xt",
                                          name=f"xt{l + 1}")
                else:
                    w1, bc1, bs1 = load_mlp_weights("Wh1", "bh1c", "bh1s")
                    w2, bc2, bs2 = load_mlp_weights("Wh2", "bh2c", "bh2s")
                    w3 = wpool.tile([BLK, KC, 1], f16, tag="w3")
                    nc.sync.dma_start(
                        w3[:],
                        t_W["Wh3pad"].ap().rearrange("(c p) n -> p c n",
                                                     p=BLK))
                    z1 = htpool.tile([BLK, KC, NP_PAD], f16, tag="ht",
                                     name="z1")
                    z2 = htpool.tile([BLK, KC, NP_PAD], f16, tag="ht",
                                     name="z2")
                    y_sb = cpool.tile([BLK, NBLK], f32)

                # prefetch block 0
                gts_cur = gather_block(l, 0, gqrr[0])
                oh_cur = stream_onehots(0)
                pend = None      # (b, gts, ohaa) awaiting aggregation
                for b in range(NBLK):
                    # prefetch the next block's gather + one-hot stream
                    # first so the GpSimd desc-gen chain runs as early as
                    # possible (it is the pacing engine).
                    if b + 1 < NBLK:
                        gts_nxt = gather_block(l, b + 1, gqrr[b + 1])
                        oh_nxt = stream_onehots(b + 1)
                    us = u_phase(l, b, xt_cur, gts_cur, oh_cur)
                    ohaa = score_phase(l, us, oh_cur)
                    # aggregate the PREVIOUS block now: its scores are long
                    # done, and the PE queue stays dense (u-matmuls of b just
                    # went in ahead of agg of b-1).
                    if pend is not None:
                        pb, pgts, pohaa = pend
                        agg_phase(l, pb, pgts, pohaa, hT_new)
                        if l < 3:
                            node_block(l + 1, hT_new, wsn, wtn, bstn,
                                       xt_next, pb)
                            if pb == NBLK // 2 - 1:
                                ag_half(l + 1, 0)
                        else:
                            mlp_block(hT_new, z1, w1, bc1, bs1, pb)
                            mlp_block(z1, z2, w2, bc2, bs2, pb)
                            yp = ptpool.tile([BLK, 1], f32, tag="s",
                                             name="yp")
                            for k in range(KC):
                                nc.tensor.matmul(
                                    yp[:], z2[:, k, pb * BLK:(pb + 1) * BLK],
                                    w3[:, k, :], start=(k == 0),
                                    stop=(k == KC - 1))
                            nc.vector.tensor_scalar(y_sb[:, pb:pb + 1], yp[:],
                                                    float(static["bh3"]),
                                                    None, OP.add)
                    pend = (b, gts_cur[0], ohaa)
                    if b + 1 < NBLK:
                        gts_cur = gts_nxt
                        oh_cur = oh_nxt
                # drain the last block
                pb, pgts, pohaa = pend
                agg_phase(l, pb, pgts, pohaa, hT_new)
                if l < 3:
                    node_block(l + 1, hT_new, wsn, wtn, bstn, xt_next, pb)
                    ag_half(l + 1, 1)
                    xt_cur = xt_next
                else:
                    mlp_block(hT_new, z1, w1, bc1, bs1, pb)
                    mlp_block(z1, z2, w2, bc2, bs2, pb)
                    yp = ptpool.tile([BLK, 1], f32, tag="s", name="yp")
                    for k in range(KC):
                        nc.tensor.matmul(
                            yp[:], z2[:, k, pb * BLK:(pb + 1) * BLK],
                            w3[:, k, :], start=(k == 0), stop=(k == KC - 1))
                    nc.vector.tensor_scalar(y_sb[:, pb:pb + 1], yp[:],
                                            float(static["bh3"]), None,
                                            OP.add)
            nc.sync.dma_start(t_y.ap(), y_sb[:])

    nc.compile()
    return nc


# ---------------- public entry points ---------------------------------------
def _get_nc(cfg, static):
    key = (tuple(sorted(cfg.items())), static["T"], tuple(static["p"]))
    if key not in _BUILD_CACHE:
        _BUILD_CACHE[key] = _build_nc(cfg, static)
    return _BUILD_CACHE[key]


def _assemble(results, cfg, slot_of):
    NC, NPC, NBLK = cfg["NC"], cfg["NPC"], cfg["NBLK"]
    NP_PAD = NBLK * BLK
    flat = np.zeros(NC * NP_PAD, np.float32)
    for cc in range(NC):
        y = np.asarray(results[cc]["y"])          # [128, NBLK]
        flat[cc * NP_PAD:(cc + 1) * NP_PAD] = y.T.reshape(-1)
    return flat[slot_of].reshape(-1, 1).astype(np.float32)


def kernel(**inputs) -> np.ndarray:
    from concourse import bass_utils
    cfg = FULL_CFG
    in_maps, static, slot_of = _host_prep(inputs, cfg)
    nc = _get_nc(cfg, static)
    res = bass_utils.run_bass_kernel_spmd(
        nc, in_maps, core_ids=list(range(cfg["NC"])), trace=False)
    return _assemble(res.results, cfg, slot_of)


def kernel_traced(**inputs):
    """Like kernel() but with NTFF profiling; returns (out, exec_time_ns)."""
    import sys, types
    if "antenv.axon_hooks" not in sys.modules:
        mod = types.ModuleType("antenv.axon_hooks")
        mod._hook = None
        mod.set_axon_ntff_profile_hook = lambda h: setattr(mod, "_hook", h)
        mod.get_axon_ntff_profile_hook = lambda: mod._hook
        sys.modules["antenv.axon_hooks"] = mod
        sys.path.insert(0, "/root/.axon_site/trn_agent_boot")
        try:
            import trn_boot
            mod._hook = trn_boot._ntff_profile_via_ctypes(
                "/opt/axon/libaxon_pjrt.so")
        except Exception as e:
            print("ntff hook unavailable:", e)
    from concourse import bass_utils
    cfg = FULL_CFG
    in_maps, static, slot_of = _host_prep(inputs, cfg)
    nc = _get_nc(cfg, static)
    res = bass_utils.run_bass_kernel_spmd(
        nc, in_maps, core_ids=list(range(cfg["NC"])), trace=True)
    return _assemble(res.results, cfg, slot_of), res.exec_time_ns
